# revision 2
# baseline (speedup 1.0000x reference)
"""Additive (Bahdanau) attention on 8 TRN2 NeuronCores.

Problem: B=8, LQ=256, LK=1024, DQ=DK=DV=512, H=128.
  q = Q @ W_q; k = K @ W_k
  scores[b,q,k] = sum_h w_v[h] * tanh(qf[b,q,h] + kf[b,k,h])
  out = softmax_k(mask(scores)) @ V

Sharding: data-parallel over batch — core i computes batch i entirely.

Per-core design (all engines explicit, Tile framework for scheduling):
  - h=H=128 lives on SBUF partitions.
  - kfT (h, LK) stays resident in PSUM (f32); qfT (h, LQ) in SBUF.
  - For each query q: one ScalarE instruction computes
        T_q = tanh(kfT + qf[:, q])   (128 x 1024, bf16, bias = per-partition)
    and two m=1 TensorE matmuls reduce over h:
        S[q, :] = w_v^T @ T_q        (1 x 512 each, into PSUM)
    Matmul PSUM rows must start at 32-aligned partitions, so rows are
    written at partitions {0,32,64,96} (4 queries per PSUM generation,
    which also makes the 4 matmuls run concurrently in distinct PE
    column groups), DVE-copied to SBUF, and gathered into compact
    partitions with a strided SBUF->SBUF DMA.
  - Masked softmax: additive mask row (0 / -50) broadcast across
    partitions once via partition-step-0 DMA; exp on ScalarE emits its
    own row sums via accum_out; normalization is deferred to the output.
  - attn @ V: PE-transpose E to get lhsT, 8 accumulating matmuls, then
    one per-partition scalar multiply by 1/rowsum.

ScalarE is the roofline: 256 tanh instructions x (172+1024) cycles
@ 1.2 GHz ~= 255 us/core; everything else overlaps underneath it.
"""

import sys

if "/opt/trn_rl_repo" not in sys.path:
    sys.path.insert(0, "/opt/trn_rl_repo")

import numpy as np
import ml_dtypes

import concourse.bass as bass
import concourse.mybir as mybir
from concourse import tile, bacc
from concourse.bass_utils import run_bass_kernel_spmd
from concourse.masks import make_identity

B, LQ, LK, DQ, DK, DV, H = 8, 256, 1024, 512, 512, 512, 128
N_CORES = 8
MASK_NEG = -50.0  # scores are bounded by sum|w_v| ~ 9, so -50 is "minus inf"

_BF16 = mybir.dt.bfloat16
_F32 = mybir.dt.float32

_cached = {}


def _build():
    nc = bacc.Bacc("TRN2", target_bir_lowering=False, debug=False)

    Qp = nc.declare_dram_parameter("Q", [LQ, DQ], _BF16, isOutput=False)
    Kp = nc.declare_dram_parameter("K", [LK, DK], _BF16, isOutput=False)
    Vp = nc.declare_dram_parameter("V", [LK, DV], _BF16, isOutput=False)
    Wqp = nc.declare_dram_parameter("Wq", [DQ, H], _BF16, isOutput=False)
    Wkp = nc.declare_dram_parameter("Wk", [DK, H], _BF16, isOutput=False)
    wvp = nc.declare_dram_parameter("wv", [H, 1], _BF16, isOutput=False)
    mp = nc.declare_dram_parameter("mask", [1, LK], _F32, isOutput=False)
    outp = nc.declare_dram_parameter("out", [LQ, DV], _F32, isOutput=True)

    NDQ = DQ // 128  # 4 contraction chunks for projections
    NKC = LK // 128  # 8 key chunks
    NQT = LQ // 128  # 2 query tiles

    with tile.TileContext(nc) as tc:
        with (
            tc.tile_pool(name="const", bufs=1) as const,
            tc.tile_pool(name="nat", bufs=3) as nat,
            tc.tile_pool(name="tpool", bufs=6) as tpool,
            tc.tile_pool(name="spool", bufs=3) as spool,
            tc.tile_pool(name="softm", bufs=2) as softm,
            tc.tile_pool(name="ps_kft", bufs=1, space="PSUM") as ps_kft,
            tc.tile_pool(name="ps_gen", bufs=2, space="PSUM") as ps_gen,
            tc.tile_pool(name="ps_misc", bufs=2, space="PSUM") as ps_misc,
        ):
            # ---- constants / weights -------------------------------------
            wq_sb = const.tile([128, NDQ, H], _BF16)
            nc.sync.dma_start(out=wq_sb, in_=Wqp[:, :].rearrange("(c p) h -> p c h", p=128))
            wk_sb = const.tile([128, NDQ, H], _BF16)
            nc.sync.dma_start(out=wk_sb, in_=Wkp[:, :].rearrange("(c p) h -> p c h", p=128))
            wv_sb = const.tile([H, 1], _BF16)
            nc.sync.dma_start(out=wv_sb, in_=wvp[:, :])
            v_sb = const.tile([128, NKC, DV], _BF16)
            nc.sync.dma_start(out=v_sb, in_=Vp[:, :].rearrange("(c p) d -> p c d", p=128))
            mask_rep = const.tile([128, LK], _F32)
            nc.sync.dma_start(out=mask_rep, in_=mp[0:1, :].to_broadcast([128, LK]))
            ident = const.tile([128, 128], _BF16)
            make_identity(nc, ident)

            # ---- transpose Q, K (PE transpose via identity) --------------
            qT_sb = const.tile([128, NDQ, LQ], _BF16)
            for qt in range(NQT):
                qn = nat.tile([128, DQ], _BF16, tag="nat")
                nc.sync.dma_start(out=qn, in_=Qp[qt * 128 : (qt + 1) * 128, :])
                pst = ps_misc.tile([128, 512], _BF16, tag="misc")
                for dc in range(NDQ):
                    nc.tensor.transpose(pst[:, dc * 128 : (dc + 1) * 128], qn[:, dc * 128 : (dc + 1) * 128], ident)
                nc.vector.tensor_copy(
                    qT_sb[:, :, qt * 128 : (qt + 1) * 128],
                    pst.rearrange("p (c x) -> p c x", c=NDQ),
                )
            kT_sb = const.tile([128, NDQ, LK], _BF16)
            for kc in range(NKC):
                kn = nat.tile([128, DK], _BF16, tag="nat")
                nc.sync.dma_start(out=kn, in_=Kp[kc * 128 : (kc + 1) * 128, :])
                pst = ps_misc.tile([128, 512], _BF16, tag="misc")
                for dc in range(NDQ):
                    nc.tensor.transpose(pst[:, dc * 128 : (dc + 1) * 128], kn[:, dc * 128 : (dc + 1) * 128], ident)
                nc.vector.tensor_copy(
                    kT_sb[:, :, kc * 128 : (kc + 1) * 128],
                    pst.rearrange("p (c x) -> p c x", c=NDQ),
                )

            # ---- projections: qfT (h, LQ) sbuf f32; kfT (h, LK) psum f32 --
            qf_ps = ps_misc.tile([128, LQ], _F32, tag="misc")
            for dc in range(NDQ):
                nc.tensor.matmul(
                    out=qf_ps,
                    lhsT=wq_sb[:, dc, :],
                    rhs=qT_sb[:, dc, :],
                    start=(dc == 0),
                    stop=(dc == NDQ - 1),
                )
            qfT_sb = const.tile([128, LQ], _F32)
            nc.vector.tensor_copy(qfT_sb, qf_ps)

            kf_ps = ps_kft.tile([128, LK], _F32)
            for half in range(2):
                for dc in range(NDQ):
                    nc.tensor.matmul(
                        out=kf_ps[:, half * 512 : (half + 1) * 512],
                        lhsT=wk_sb[:, dc, :],
                        rhs=kT_sb[:, dc, half * 512 : half * 512 + 512],
                        start=(dc == 0),
                        stop=(dc == NDQ - 1),
                    )

            # ---- main loop: scores -> softmax -> attn @ V ----------------
            for qt in range(NQT):
                s_sb = spool.tile([128, LK], _F32, tag="s")
                # 32 generations of 4 queries each
                for g in range(32):
                    sg = ps_gen.tile([128, LK], _F32)
                    for j in range(4):
                        q = qt * 128 + g * 4 + j
                        p = 32 * j
                        t_q = tpool.tile([128, LK], _BF16, tag="t")
                        nc.scalar.activation(
                            out=t_q,
                            in_=kf_ps,
                            func=mybir.ActivationFunctionType.Tanh,
                            bias=qfT_sb[:, q : q + 1],
                            scale=1.0,
                        )
                        for half in range(2):
                            nc.tensor.matmul(
                                out=sg[p : p + 1, half * 512 : (half + 1) * 512],
                                lhsT=wv_sb,
                                rhs=t_q[:, half * 512 : half * 512 + 512],
                                start=True,
                                stop=True,
                                skip_group_check=True,
                                tile_position=(0, p),
                            )
                    stg = spool.tile([128, LK], _F32, tag="stg")
                    nc.vector.tensor_copy(stg, sg)
                    nc.sync.dma_start(out=s_sb[g * 4 : g * 4 + 4, :], in_=stg[0:128:32, :])

                # mask + softmax (unnormalized; rowsum via accum_out)
                sm = softm.tile([128, LK], _F32, tag="sm")
                nc.vector.tensor_add(sm, s_sb, mask_rep)
                e_sb = softm.tile([128, LK], _BF16, tag="e")
                rsum = softm.tile([128, 1], _F32, tag="rs")
                nc.scalar.activation(
                    out=e_sb,
                    in_=sm,
                    func=mybir.ActivationFunctionType.Exp,
                    accum_out=rsum,
                )
                rinv = softm.tile([128, 1], _F32, tag="ri")
                nc.vector.reciprocal(rinv, rsum)

                # transpose E -> lhsT chunks
                eT_sb = softm.tile([128, LK], _BF16, tag="et")
                for gtr in range(2):
                    pst = ps_misc.tile([128, 512], _BF16, tag="misc")
                    for j in range(4):
                        kc = gtr * 4 + j
                        nc.tensor.transpose(pst[:, j * 128 : (j + 1) * 128], e_sb[:, kc * 128 : (kc + 1) * 128], ident)
                    nc.vector.tensor_copy(eT_sb[:, gtr * 512 : (gtr + 1) * 512], pst)

                # attn @ V with deferred normalization
                o_ps = ps_misc.tile([128, DV], _F32, tag="misc")
                for kc in range(NKC):
                    nc.tensor.matmul(
                        out=o_ps,
                        lhsT=eT_sb[:, kc * 128 : (kc + 1) * 128],
                        rhs=v_sb[:, kc, :],
                        start=(kc == 0),
                        stop=(kc == NKC - 1),
                    )
                osb = softm.tile([128, DV], _F32, tag="o")
                nc.vector.tensor_scalar_mul(osb, o_ps, rinv[:, 0:1])
                nc.sync.dma_start(out=outp[qt * 128 : (qt + 1) * 128, :], in_=osb)

    nc.finalize()
    return nc


def _get_nc():
    if "nc" not in _cached:
        _cached["nc"] = _build()
    return _cached["nc"]


def kernel(Q, K, V, valid_lengths, W_q, W_k, w_v, _want_trace=False, _trace_kwargs=None):
    nc = _get_nc()

    Q = np.asarray(Q, dtype=np.float32)
    K = np.asarray(K, dtype=np.float32)
    V = np.asarray(V, dtype=np.float32)
    vl = np.asarray(valid_lengths)
    W_q = np.asarray(W_q, dtype=np.float32)
    W_k = np.asarray(W_k, dtype=np.float32)
    w_v = np.asarray(w_v, dtype=np.float32)

    bf = ml_dtypes.bfloat16
    Qb = Q.astype(bf)
    Kb = K.astype(bf)
    Vb = V.astype(bf)
    Wqb = W_q.astype(bf)
    Wkb = W_k.astype(bf)
    wvb = w_v.reshape(H, 1).astype(bf)
    # additive mask rows per batch: 0 where key valid, -50 where masked
    mask = np.where(
        np.arange(LK)[None, :] < vl.reshape(B, 1).astype(np.int64),
        np.float32(0.0),
        np.float32(MASK_NEG),
    ).astype(np.float32)

    in_maps = [
        {
            "Q": Qb[i],
            "K": Kb[i],
            "V": Vb[i],
            "Wq": Wqb,
            "Wk": Wkb,
            "wv": wvb,
            "mask": mask[i : i + 1],
        }
        for i in range(N_CORES)
    ]

    kwargs = {}
    if _want_trace:
        kwargs["trace"] = True
        if _trace_kwargs:
            kwargs.update(_trace_kwargs)
    res = run_bass_kernel_spmd(nc, in_maps, core_ids=list(range(N_CORES)), **kwargs)
    out = np.stack([res.results[i]["out"] for i in range(N_CORES)], axis=0)
    if _want_trace:
        _cached["last_result"] = res
    return out


# revision 18
# speedup vs baseline: 17.8588x; 17.8588x over previous
"""Additive (Bahdanau) attention on 8 TRN2 NeuronCores.

Problem: B=8, LQ=256, LK=1024, DQ=DK=DV=512, H=128.
  q = Q @ W_q; k = K @ W_k
  scores[b,q,k] = sum_h w_v[h] * tanh(qf[b,q,h] + kf[b,k,h])
  out = softmax_k(mask(scores)) @ V

Sharding: data-parallel over QUERIES — core c computes query rows
[32c, 32c+32) of every batch. Each batch's key range is statically
trimmed to extent_b = ceil(valid_len_b / 128) * 128 (the kernel graph
is built per call from the actual valid_lengths, so this stays fully
general): masked keys beyond the extent contribute exactly zero
attention, so they are never computed. Every core's workload is
identically sum_b 32 * extent_b -> perfect balance regardless of the
length distribution, with no cross-core communication (softmax rows
live entirely on one core).

Per-core pipeline (h=H=128 on SBUF partitions):
  - Per batch: K rows [0, extent_b) are PE-transposed and projected;
    kfT_b is DVE-copied to SBUF so PSUM stays free for the score
    generations and later batches' K paths overlap the tanh stream.
  - Per query: one ScalarE tanh instruction (T = tanh(kfT + qf_col),
    FD = extent_b) and ceil(extent/512) m=1 TensorE matmuls reduce
    over h with stationary w_v. Matmul PSUM rows must start at
    32-aligned partitions, so 4 queries per PSUM generation land at
    partitions {0,32,64,96} (concurrent PE column groups), are
    DVE-copied to SBUF and gathered into compact partitions by a
    strided SBUF->SBUF DMA.
  - Masked softmax: additive mask row (0 / -50) broadcast over 32
    partitions by a partition-step-0 DMA; exp on ScalarE emits row
    sums via accum_out; normalization deferred to the output.
    The softmax/AV epilogue of batch b is issued after batch b+1's
    score loop so the ScalarE FIFO never stalls on the gather DMAs.
  - attn @ V over valid key chunks only; per-partition multiply by
    1/rowsum; DMA out.

ScalarE is the roofline: sum_b 32*(224+extent_b) cycles @ 1.2 GHz.
"""

import sys

if "/opt/trn_rl_repo" not in sys.path:
    sys.path.insert(0, "/opt/trn_rl_repo")

import numpy as np
import ml_dtypes

import concourse.bass as bass
import concourse.mybir as mybir
from concourse import tile, bacc
from concourse.bass_utils import run_bass_kernel_spmd
B, LQ, LK, DQ, DK, DV, H = 8, 256, 1024, 512, 512, 512, 128
N_CORES = 8
QPC = LQ // N_CORES  # 32 query rows per core per batch
MASK_NEG = -50.0  # scores bounded by sum|w_v| ~ 9, so -50 is "minus inf"

_BF16 = mybir.dt.bfloat16
_F32 = mybir.dt.float32

# Degree-9 odd minimax-ish polynomial for tanh on [-3.6, 3.6] (max err
# ~0.012; inputs are clamped to that range first, clamp error <= 1.5e-3).
# Used only for the small fraction of queries whose tanh is offloaded
# from ScalarE to the otherwise-idle VectorE.
_TANH_CLAMP = 3.6
_TANH_C = (0.95397023, -0.21573944, 0.032842446, -0.0024608947, 6.959084e-05)

_cached = {}


def _plan_offload(extents):
    """Pick which (batch, query) pairs compute tanh on VectorE so ACT and
    DVE busy-times balance (with margin). Returns set of (b, i)."""
    act = {b: (222 + e) / 1.2 for b, e in enumerate(extents)}  # ns per query
    dve = {b: (4.0 * e + 11 * 58) / 0.96 for b, e in enumerate(extents)}
    act_total = sum(32 * act[b] for b in range(len(extents))) + 8000.0  # + exps
    # baseline DVE busy: gather copies + kf/qT copies + mask adds (approx)
    dve_total = sum(8 * (120 + e) / 0.96 for e in extents) + 40000.0
    off = set()
    margin = 14000.0
    i_per_b = {b: 0 for b in range(len(extents))}
    while True:
        cand = min(range(len(extents)), key=lambda b: (i_per_b[b], -extents[b]))
        b = cand
        if i_per_b[b] >= 16:
            break
        new_act = act_total - act[b]
        new_dve = dve_total + dve[b]
        if new_dve + margin >= new_act:
            break
        perm = (0, 8, 16, 24, 4, 12, 20, 28, 2, 10, 18, 26, 6, 14, 22, 30)
        off.add((b, perm[i_per_b[b]]))  # spread offloads across generations
        i_per_b[b] += 1
        act_total, dve_total = new_act, new_dve
    return off


def _build(extents):
    """Build the SPMD graph for one core given per-batch key extents
    (each a multiple of 128, in [128, 1024])."""
    nc = bacc.Bacc("TRN2", target_bir_lowering=False, debug=False)

    total_k = int(sum(extents))
    Qp = nc.declare_dram_parameter("Q", [B * QPC, DQ], _BF16, isOutput=False)
    Kp = nc.declare_dram_parameter("K", [total_k, DK], _BF16, isOutput=False)
    Vp = nc.declare_dram_parameter("V", [total_k, DV], _BF16, isOutput=False)
    Wqp = nc.declare_dram_parameter("Wq", [DQ, H], _BF16, isOutput=False)
    Wkp = nc.declare_dram_parameter("Wk", [DK, H], _BF16, isOutput=False)
    wvp = nc.declare_dram_parameter("wv", [H, 1], _BF16, isOutput=False)
    idp = nc.declare_dram_parameter("ident", [128, 128], _BF16, isOutput=False)
    mp = nc.declare_dram_parameter("mask", [B, LK], _F32, isOutput=False)
    outp = nc.declare_dram_parameter("out", [B, QPC, DV], _F32, isOutput=True)

    NDQ = DQ // 128  # 4 contraction chunks for the projections
    offs = np.concatenate([[0], np.cumsum(extents)]).astype(int)
    offload = _plan_offload(extents)

    with tile.TileContext(nc) as tc:
        with (
            tc.tile_pool(name="const", bufs=1) as const,
            tc.tile_pool(name="nat", bufs=4) as nat,
            tc.tile_pool(name="kv", bufs=3) as kv,
            tc.tile_pool(name="tpool", bufs=10) as tpool,
            tc.tile_pool(name="tpoly", bufs=2) as tpoly,
            tc.tile_pool(name="spool", bufs=4) as spool,
            tc.tile_pool(name="softm", bufs=2) as softm,
            tc.tile_pool(name="ps_gen", bufs=2, space="PSUM") as ps_gen,
            tc.tile_pool(name="ps_kp", bufs=2, space="PSUM") as ps_kp,
            tc.tile_pool(name="ps_tail", bufs=2, space="PSUM") as ps_tail,
        ):
            # ---- constants / weights -------------------------------------
            wq_sb = const.tile([128, NDQ, H], _BF16)
            nc.gpsimd.dma_start(out=wq_sb, in_=Wqp[:, :].rearrange("(c p) h -> p c h", p=128))
            wk_sb = const.tile([128, NDQ, H], _BF16)
            nc.gpsimd.dma_start(out=wk_sb, in_=Wkp[:, :].rearrange("(c p) h -> p c h", p=128))
            wv_sb = const.tile([H, 1], _BF16)
            nc.gpsimd.dma_start(out=wv_sb, in_=wvp[:, :])
            ident = const.tile([128, 128], _BF16)
            nc.sync.dma_start(out=ident, in_=idp[:, :])

            # ---- qfT (h, B*QPC) for this core's queries ------------------
            qT_sb = const.tile([128, NDQ, B * QPC], _BF16)
            for qt in range(B * QPC // 128):
                qn = nat.tile([128, DQ], _BF16, tag="nat")
                nc.sync.dma_start(out=qn, in_=Qp[qt * 128 : (qt + 1) * 128, :])
                pst = ps_kp.tile([128, 512], _BF16, tag="kp")
                for dc in range(NDQ):
                    nc.tensor.transpose(pst[:, dc * 128 : (dc + 1) * 128], qn[:, dc * 128 : (dc + 1) * 128], ident)
                nc.vector.tensor_copy(
                    qT_sb[:, :, qt * 128 : (qt + 1) * 128],
                    pst.rearrange("p (c x) -> p c x", c=NDQ),
                )
            qf_ps = ps_tail.tile([128, B * QPC], _F32, tag="tail")
            for dc in range(NDQ):
                nc.tensor.matmul(
                    out=qf_ps,
                    lhsT=wq_sb[:, dc, :],
                    rhs=qT_sb[:, dc, :],
                    start=(dc == 0),
                    stop=(dc == NDQ - 1),
                )
            qfT_sb = const.tile([128, B * QPC], _F32)
            nc.vector.tensor_copy(qfT_sb, qf_ps)

            # ---- helpers --------------------------------------------------
            def k_path(b, first=False):
                """K transpose + projection; kfT_b lands in SBUF (f32)."""
                ext = int(extents[b])
                nkc = ext // 128
                o0 = int(offs[b])
                kT_b = kv.tile([128, NDQ, ext], _BF16, tag="kT")
                for kc in range(nkc):
                    kn = nat.tile([128, DK], _BF16, tag="nat")
                    nc.sync.dma_start(out=kn, in_=Kp[o0 + kc * 128 : o0 + (kc + 1) * 128, :])
                    pst = ps_kp.tile([128, 512], _BF16, tag="kp")
                    for dc in range(NDQ):
                        nc.tensor.transpose(pst[:, dc * 128 : (dc + 1) * 128], kn[:, dc * 128 : (dc + 1) * 128], ident)
                    nc.vector.tensor_copy(
                        kT_b[:, :, kc * 128 : (kc + 1) * 128],
                        pst.rearrange("p (c x) -> p c x", c=NDQ),
                    )
                kf_sb = kv.tile([128, ext], _F32, tag="kf")
                for c0 in range(0, ext, 512):
                    cn = min(512, ext - c0)
                    kf_ps = ps_kp.tile([128, 512], _F32, tag="kp")
                    for dc in range(NDQ):
                        nc.tensor.matmul(
                            out=kf_ps[:, 0:cn],
                            lhsT=wk_sb[:, dc, :],
                            rhs=kT_b[:, dc, c0 : c0 + cn],
                            start=(dc == 0),
                            stop=(dc == NDQ - 1),
                        )
                    nc.vector.tensor_copy(kf_sb[:, c0 : c0 + cn], kf_ps[:, 0:cn])
                v_b = kv.tile([128, nkc, DV], _BF16, tag="v")
                nc.gpsimd.dma_start(
                    out=v_b, in_=Vp[o0 : o0 + ext, :].rearrange("(c p) d -> p c d", p=128)
                )
                mask_b = kv.tile([QPC, ext], _F32, tag="mb")
                nc.gpsimd.dma_start(out=mask_b, in_=mp[b : b + 1, 0:ext].to_broadcast([QPC, ext]))
                return kf_sb, v_b, mask_b

            def scores(b, kf_sb):
                """tanh + m=1 reduce matmuls + gather for batch b."""
                ext = int(extents[b])
                nchunks = [(c0, min(512, ext - c0)) for c0 in range(0, ext, 512)]
                s_b = spool.tile([QPC, ext], _F32, tag="s")
                for g in range(QPC // 4):
                    sg = ps_gen.tile([128, ext], _F32, tag="gen")
                    for j in range(4):
                        q = b * QPC + g * 4 + j
                        p = 32 * j
                        t_q = tpool.tile([128, ext], _BF16, tag="t")
                        if (b, g * 4 + j) in offload:
                            # polynomial tanh on VectorE (frees ScalarE time)
                            AL = mybir.AluOpType
                            c0_, c1_, c2_, c3_, c4_ = _TANH_C
                            tx = tpoly.tile([128, ext], _BF16, tag="tx")
                            nc.vector.tensor_scalar(
                                out=tx, in0=kf_sb, scalar1=qfT_sb[:, q : q + 1],
                                scalar2=_TANH_CLAMP, op0=AL.add, op1=AL.min,
                            )
                            nc.vector.tensor_scalar(
                                out=tx, in0=tx, scalar1=-_TANH_CLAMP, scalar2=None,
                                op0=AL.max,
                            )
                            tu = tpoly.tile([128, ext], _BF16, tag="tu")
                            nc.vector.tensor_tensor(out=tu, in0=tx, in1=tx, op=AL.mult)
                            tw = tpoly.tile([128, ext], _BF16, tag="tw")
                            nc.vector.tensor_scalar(
                                out=tw, in0=tu, scalar1=c4_, scalar2=None, op0=AL.mult
                            )
                            for cc in (c3_, c2_, c1_):
                                nc.vector.tensor_scalar(
                                    out=tw, in0=tw, scalar1=cc, scalar2=None, op0=AL.add
                                )
                                nc.vector.tensor_tensor(out=tw, in0=tw, in1=tu, op=AL.mult)
                            nc.vector.tensor_scalar(
                                out=tw, in0=tw, scalar1=c0_, scalar2=None, op0=AL.add
                            )
                            nc.vector.tensor_tensor(out=t_q, in0=tw, in1=tx, op=AL.mult)
                        else:
                            nc.scalar.activation(
                                out=t_q,
                                in_=kf_sb,
                                func=mybir.ActivationFunctionType.Tanh,
                                bias=qfT_sb[:, q : q + 1],
                                scale=1.0,
                            )
                        for c0, cn in nchunks:
                            nc.tensor.matmul(
                                out=sg[p : p + 1, c0 : c0 + cn],
                                lhsT=wv_sb,
                                rhs=t_q[:, c0 : c0 + cn],
                                start=True,
                                stop=True,
                                skip_group_check=True,
                                tile_position=(0, p),
                            )
                    stg = spool.tile([128, ext], _F32, tag="stg")
                    nc.vector.tensor_copy(stg, sg)
                    nc.gpsimd.dma_start(out=s_b[g * 4 : g * 4 + 4, :], in_=stg[0:128:32, :])
                return s_b

            def epilogue(b, s_b, v_b, mask_b):
                """masked softmax + attn @ V + output DMA for batch b."""
                ext = int(extents[b])
                nkc = ext // 128
                sm = softm.tile([QPC, ext], _F32, tag="sm")
                nc.vector.tensor_add(sm, s_b, mask_b)
                e_b = softm.tile([QPC, ext], _BF16, tag="e")
                rsum = softm.tile([QPC, 1], _F32, tag="rs")
                nc.scalar.activation(
                    out=e_b, in_=sm, func=mybir.ActivationFunctionType.Exp, accum_out=rsum
                )
                rinv = softm.tile([QPC, 1], _F32, tag="ri")
                nc.vector.reciprocal(rinv, rsum)

                eT_b = softm.tile([128, nkc * QPC], _BF16, tag="et")
                for g4 in range(0, nkc, 4):
                    gn = min(4, nkc - g4)
                    pst = ps_tail.tile([128, 4 * QPC], _BF16, tag="tail")
                    for j in range(gn):
                        kc = g4 + j
                        nc.tensor.transpose(
                            pst[:, j * QPC : (j + 1) * QPC],
                            e_b[:, kc * 128 : (kc + 1) * 128],
                            ident[0:QPC, 0:QPC],
                        )
                    nc.vector.tensor_copy(
                        eT_b[:, g4 * QPC : (g4 + gn) * QPC], pst[:, 0 : gn * QPC]
                    )

                o_ps = ps_tail.tile([QPC, DV], _F32, tag="tail")
                for kc in range(nkc):
                    nc.tensor.matmul(
                        out=o_ps,
                        lhsT=eT_b[:, kc * QPC : (kc + 1) * QPC],
                        rhs=v_b[:, kc, :],
                        start=(kc == 0),
                        stop=(kc == nkc - 1),
                    )
                osb = softm.tile([QPC, DV], _F32, tag="o")
                nc.vector.tensor_scalar_mul(osb, o_ps, rinv[:, 0:1])
                nc.sync.dma_start(out=outp[b, :, :], in_=osb)

            # ---- software-pipelined batch loop ---------------------------
            # epilogue(b) is issued after scores(b+1) so ScalarE's exp never
            # blocks the next batch's tanh stream waiting on gather DMAs.
            # Batch order: 2nd-smallest first (short pipeline fill), smallest
            # last (short drain: its gather chain and epilogue set the tail).
            asc = sorted(range(B), key=lambda b: (int(extents[b]), b))
            batch_order = [asc[1]] + asc[2:][::-1] + [asc[0]]
            pending = None
            for bi, b in enumerate(batch_order):
                kf_sb, v_b, mask_b = k_path(b, first=(bi == 0))
                s_b = scores(b, kf_sb)
                if pending is not None:
                    epilogue(*pending)
                pending = (b, s_b, v_b, mask_b)
            epilogue(*pending)

    nc.finalize()
    return nc


def _get_nc(extents):
    key = tuple(int(e) for e in extents)
    if key not in _cached:
        _cached[key] = _build(key)
    return _cached[key]


def kernel(Q, K, V, valid_lengths, W_q, W_k, w_v, _want_trace=False):
    Q = np.asarray(Q, dtype=np.float32)
    K = np.asarray(K, dtype=np.float32)
    V = np.asarray(V, dtype=np.float32)
    vl = np.asarray(valid_lengths).astype(np.int64).reshape(B)
    W_q = np.asarray(W_q, dtype=np.float32)
    W_k = np.asarray(W_k, dtype=np.float32)
    w_v = np.asarray(w_v, dtype=np.float32)

    extents = np.clip(np.ceil(vl / 128.0).astype(int) * 128, 128, LK)
    nc = _get_nc(extents)

    bf = ml_dtypes.bfloat16
    Kc = np.concatenate([K[b, : extents[b], :] for b in range(B)], axis=0).astype(bf)
    Vc = np.concatenate([V[b, : extents[b], :] for b in range(B)], axis=0).astype(bf)
    Wqb = W_q.astype(bf)
    Wkb = W_k.astype(bf)
    wvb = w_v.reshape(H, 1).astype(bf)
    mask = np.where(
        np.arange(LK)[None, :] < vl[:, None], np.float32(0.0), np.float32(MASK_NEG)
    ).astype(np.float32)
    Qb = Q.astype(bf)

    in_maps = []
    for c in range(N_CORES):
        Qcore = np.concatenate(
            [Qb[b, c * QPC : (c + 1) * QPC, :] for b in range(B)], axis=0
        )
        in_maps.append(
            {
                "Q": Qcore,
                "K": Kc,
                "V": Vc,
                "Wq": Wqb,
                "Wk": Wkb,
                "wv": wvb,
                "mask": mask,
                "ident": np.eye(128, dtype=bf),
            }
        )

    kwargs = {"trace": True} if _want_trace else {}
    res = run_bass_kernel_spmd(nc, in_maps, core_ids=list(range(N_CORES)), **kwargs)
    out = np.empty((B, LQ, DV), dtype=np.float32)
    for c in range(N_CORES):
        oc = res.results[c]["out"]  # (B, QPC, DV)
        for b in range(B):
            out[b, c * QPC : (c + 1) * QPC, :] = oc[b]
    if _want_trace:
        _cached["last_result"] = res
    return out
